# revision 19
# baseline (speedup 1.0000x reference)
"""XNOR-Net conv2d kernel for Trainium2.

Computes conv2d(sign(x), sign(W), stride=1, pad=1) * alpha for
x:(32,256,56,56) f32, W:(256,256,3,3) f32, alpha:(256,1,1) f32.

Strategy: data-parallel over batch (4 images per core x 8 cores).
Per core, implicit GEMM on the PE array in fp8. sign(x) is +-1 in
fp8e4 (exact); sign(W) is represented as +-0.5 (one-pass compute:
(w>0) - 0.5), with the missing x2 folded into alpha. Products are
+-0.5, accumulated in fp32 PSUM -> half-integers, exact; the final
scale restores integers, so the result is bit-exact vs the reference.

sign(x) lives in SBUF as a zero-padded fp8 image
[128 part = C_in%128, 2 c-groups, 58 rows, 64 row-stride]. Each 3x3
tap is one DoubleRow matmul contracting all 256 input channels
(K = 128 partitions x 2 c-groups): lhsT [128, 2cg, 128co], rhs
[128, 2cg, 8 rows, 56 cols] (shifted window, N=448). 9 taps
accumulate into one PSUM bank; copyback applies 2*alpha.

v9 schedule (vs ~122-127us for earlier versions):
- Strict load priority via a single HWDGE ring: ALL loads (weights,
  x) ride the scalar-engine queue in exact FIFO priority order
  [w-mt0cg0, alpha, w-mt0cg1, x rows 0-8, x rows 9-40, w-mt1cg0,
  x rows 41-48, w-mt1cg1, x rows 49-55, img1...]. Splitting loads
  across the two HWDGE rings lets the SDMA engines round-robin
  ~1:1 between rings, which starves whichever matters now; one ring
  makes priority deterministic. ALL y stores ride the sync ring, so
  stores never queue behind loads and the ring is warm at the tail.
- Startup weight fast path: PE-transposes the raw fp32 weights the
  moment each DMA lands (no SBUF sign stage) and folds (w>0)-0.5
  into the PSUM->SBUF evacuation on the DVE. First real matmul at
  ~13us with the HAM clock gate already at 8/8 from warmup matmuls,
  and the PE never idles long enough to re-throttle.
- mt1 weight prep (bf16 sign->transpose->cast path) is interleaved
  into img0's mt0 row-groups to match its DMA arrival (~20-22us).
- Tail: the final row-group computes mt0, then mt1 as 6-row + 2-row
  PSUM groups; the last 57KB store issues on the already-spinning
  sync ring right after a 2-row copyback, cutting the post-last-
  matmul tail to ~3.5us (store completion receipt dominates).
"""

import sys

sys.path.insert(0, "/opt/trn_rl_repo")

import numpy as np

import concourse.bass as bass
import concourse.mybir as mybir
from concourse import bacc
from concourse.bass_utils import run_bass_kernel_spmd
from concourse.masks import make_identity
from concourse.tile import TileContext

P = 128
N_CORES = 8
N_IMG = 32
IMG_PER_CORE = N_IMG // N_CORES
C = 256
H = W = 56
HP = 58  # padded rows (0..57)
WS = 64  # row stride of padded buffer (cols 0..57 used, 58+ never read)
CHUNK = 8  # output rows per matmul tile -> N = 8*56 = 448
LCHUNK = 16  # max rows per x load DMA
# (row0, nrows) per load DMA for steady-state images
CHUNKS = [(0, 9), (9, 16), (25, 16), (41, 15)]
FP8 = mybir.dt.float8e4

last_result = None  # stash of BassKernelResults for test harnesses


def build_conv_kernel():
    nc = bacc.Bacc()
    x_in = nc.declare_dram_parameter(
        "x", [IMG_PER_CORE, C, H, W], mybir.dt.float32, isOutput=False
    )
    w_in = nc.declare_dram_parameter("w", [C, C, 3, 3], mybir.dt.float32, isOutput=False)
    a_in = nc.declare_dram_parameter("alpha", [C, 1, 1], mybir.dt.float32, isOutput=False)
    y_out = nc.declare_dram_parameter(
        "y", [IMG_PER_CORE, C, H, W], mybir.dt.float32, isOutput=True
    )
    x_ap, w_ap, a_ap, y_ap = x_in[:], w_in[:], a_in[:], y_out[:]

    with TileContext(nc) as tc:
        with (
            tc.tile_pool(name="wpool", bufs=1) as wpool,
            tc.tile_pool(name="xpool", bufs=3) as xpool,
            tc.tile_pool(name="opool", bufs=8) as opool,
            tc.tile_pool(name="pp", bufs=4, space="PSUM") as pp,
        ):
            # PE prewarm: matmuls over zeros, issued before any real
            # dependency, so the clock gate is at 8/8 when weights land
            warm_rhs = wpool.tile([P, 512], FP8, name="warm_rhs")
            nc.vector.memset(warm_rhs, 0.0)
            warm_acc = pp.tile([P, 512], mybir.dt.float32, name="warm_acc", bufs=1)

            def emit_warm(n):
                for _ in range(n):
                    nc.tensor.matmul(
                        warm_acc, warm_rhs[:, 0:P], warm_rhs, start=True, stop=True
                    )

            emit_warm(7)

            # warm up the ACT function table while the first DMAs run
            warm = wpool.tile([P, 1], mybir.dt.float32, name="warm")
            nc.vector.memset(warm, 0.0)
            nc.scalar.sign(warm, warm)

            ident_f32 = wpool.tile([P, P], mybir.dt.float32, name="ident_f32")
            make_identity(nc, ident_f32)
            ident = wpool.tile([P, P], mybir.dt.bfloat16, name="ident")
            make_identity(nc, ident)
            alpha_sb = wpool.tile([P, 2], mybir.dt.float32, name="alpha_sb")

            # [ci_lo, cg, mt, pos, co]
            w_lhsT = wpool.tile([P, 2, 2, 9, P], FP8, name="w_lhsT")

            wsrcs = {}
            wsgns = {}

            def emit_wdma(mt, cg):
                wsrc = wpool.tile(
                    [P, P, 9], mybir.dt.float32, name=f"wsrc{mt}", bufs=2
                )
                nc.scalar.dma_start(
                    out=wsrc,
                    in_=w_ap[
                        mt * P : (mt + 1) * P, cg * P : (cg + 1) * P
                    ].rearrange("co ci kh kw -> co ci (kh kw)"),
                )
                wsrcs[(mt, cg)] = wsrc

            def emit_wsign(mt, cg):
                # one-pass half-sign on DVE: (w > 0) - 0.5 -> +-0.5
                wsgn = wpool.tile(
                    [P, P, 9], mybir.dt.bfloat16, name=f"wsgn{mt}", bufs=2
                )
                nc.vector.tensor_scalar(
                    out=wsgn,
                    in0=wsrcs[(mt, cg)],
                    scalar1=0.0,
                    scalar2=0.5,
                    op0=mybir.AluOpType.is_gt,
                    op1=mybir.AluOpType.subtract,
                )
                wsgns[(mt, cg)] = wsgn

            def emit_wtrans(mt, cg, tri, cast_on_scalar=False):
                # transpose taps 3*tri..3*tri+2 into one PSUM tile, then a
                # single cast moves all three into the fp8 lhsT
                tp = pp.tile([P, 3, P], mybir.dt.bfloat16, name="tp", bufs=2)
                for k in range(3):
                    nc.tensor.transpose(
                        tp[:, k, :], wsgns[(mt, cg)][:, :, 3 * tri + k], ident
                    )
                dst = w_lhsT[:, cg, mt, 3 * tri : 3 * tri + 3, :]
                if cast_on_scalar:
                    nc.scalar.copy(out=dst, in_=tp)
                else:
                    nc.vector.tensor_copy(out=dst, in_=tp)

            def emit_wtrans_f32(mt, cg, tri):
                # startup fast path: PE-transpose the raw fp32 weights the
                # moment their DMA lands (no wsgn stage), and fold the
                # half-sign (w>0)-0.5 into the PSUM->SBUF evacuation
                tpf = pp.tile([P, 3, P], mybir.dt.float32, name="tpf", bufs=2)
                for k in range(3):
                    nc.tensor.transpose(
                        tpf[:, k, :], wsrcs[(mt, cg)][:, :, 3 * tri + k], ident_f32
                    )
                nc.vector.tensor_scalar(
                    out=w_lhsT[:, cg, mt, 3 * tri : 3 * tri + 3, :],
                    in0=tpf,
                    scalar1=0.0,
                    scalar2=0.5,
                    op0=mybir.AluOpType.is_gt,
                    op1=mybir.AluOpType.subtract,
                )

            xpads = {}

            def emit_xpad(img):
                xpad = xpool.tile([P, 2, HP, WS], FP8, name="xpad")
                xpads[img] = xpad
                nc.vector.memset(xpad[:, :, 0, 0:58], 0.0)
                nc.vector.memset(xpad[:, :, HP - 1, 0:58], 0.0)
                nc.vector.memset(xpad[:, :, 1 : HP - 1, 0], 0.0)
                nc.vector.memset(xpad[:, :, 1 : HP - 1, 57], 0.0)

            def emit_loads(img, chunks=None):
                # all x loads ride the scalar (load) ring
                if chunks is None:
                    emit_xpad(img)
                    chunks = CHUNKS
                srcs = []
                for r0, rows in chunks:
                    for cg in range(2):
                        xsrc = xpool.tile(
                            [P, LCHUNK, W], mybir.dt.float32, name="xsrc", bufs=10
                        )
                        nc.scalar.dma_start(
                            out=xsrc[:, 0:rows, :],
                            in_=x_ap[img, cg * P : (cg + 1) * P, r0 : r0 + rows],
                        )
                        srcs.append((r0, rows, cg, xsrc))
                return srcs

            def emit_signs(img, srcs, split=False):
                # sign on the ACT engine; split=True signs a 16-row chunk
                # as two 8-row ops so a row-group never waits on rows it
                # doesn't need yet
                xpad = xpads[img]
                for r0, rows, cg, xsrc in srcs:
                    pieces = (
                        [(0, rows // 2), (rows // 2, rows - rows // 2)]
                        if split and rows > 8
                        else [(0, rows)]
                    )
                    for p0, pr in pieces:
                        nc.scalar.sign(
                            xpad[
                                :, cg, r0 + p0 + 1 : r0 + p0 + 1 + pr, 1 : W + 1
                            ],
                            xsrc[:, p0 : p0 + pr, :],
                        )

            def emit_mm_group(img, h0, mt, ot, r0=0, nrows=CHUNK):
                # h0: absolute first output row; result rows land in
                # ot[:, mt, r0:r0+nrows]
                xpad = xpads[img]
                acc = pp.tile([P, nrows * W], mybir.dt.float32, name="acc", bufs=3)
                k = 0
                for kh in range(3):
                    for kw in range(3):
                        nc.tensor.matmul(
                            acc,
                            w_lhsT[:, :, mt, kh * 3 + kw, :],
                            xpad[:, :, h0 + kh : h0 + kh + nrows, kw : kw + W],
                            start=(k == 0),
                            stop=(k == 8),
                            perf_mode=mybir.MatmulPerfMode.DoubleRow,
                        )
                        k += 1
                # x2 restores the +-0.5 weight scale
                nc.vector.tensor_scalar(
                    out=ot[:, mt, r0 : r0 + nrows],
                    in0=acc.rearrange("p (r c) -> p r c", c=W),
                    scalar1=alpha_sb[:, mt : mt + 1],
                    scalar2=2.0,
                    op0=mybir.AluOpType.mult,
                    op1=mybir.AluOpType.mult,
                )

            def emit_row_group(img, h0):
                # both output halves for rows h0..h0+8, then one store on
                # the sync (store) ring
                ot = opool.tile([P, 2, CHUNK, W], mybir.dt.float32, name="ot")
                ydst = y_ap[img].rearrange("(mt c) h w -> c mt h w", mt=2)[
                    :, :, h0 : h0 + CHUNK, :
                ]
                emit_mm_group(img, h0, 0, ot)
                emit_mm_group(img, h0, 1, ot)
                nc.sync.dma_start(out=ydst, in_=ot)

            def emit_tail_group(img, h0):
                # final row-group: mt0 stored as soon as its copyback is
                # done; mt1 split into 6-row + 2-row PSUM groups so the
                # first store's completion receipt overlaps the last rows'
                # matmuls
                ot = opool.tile([P, 2, CHUNK, W], mybir.dt.float32, name="ot")
                ydst = y_ap[img].rearrange("(mt c) h w -> c mt h w", mt=2)[
                    :, :, h0 : h0 + CHUNK, :
                ]
                emit_mm_group(img, h0, 0, ot)
                nc.sync.dma_start(out=ydst[:, 0:1], in_=ot[:, 0:1])
                emit_mm_group(img, h0, 1, ot, r0=0, nrows=6)
                nc.sync.dma_start(out=ydst[:, 1:2, 0:6], in_=ot[:, 1:2, 0:6])
                emit_mm_group(img, h0 + 6, 1, ot, r0=6, nrows=2)
                nc.sync.dma_start(out=ydst[:, 1:2, 6:8], in_=ot[:, 1:2, 6:8])

            def emit_mms(img):
                last_img = img == IMG_PER_CORE - 1
                for h0 in range(0, H, CHUNK):
                    if last_img and h0 == H - CHUNK:
                        emit_tail_group(img, h0)
                    else:
                        emit_row_group(img, h0)

            # ---- startup: one load ring, strict priority order ----
            with tc.high_priority():
                emit_wdma(0, 0)
                nc.scalar.dma_start(
                    out=alpha_sb,
                    in_=a_ap.flatten().rearrange("(mt co) -> co mt", co=P),
                )
                emit_wdma(0, 1)
            emit_xpad(0)
            s = emit_loads(0, chunks=[(0, 9)])
            emit_signs(0, s)
            s = emit_loads(0, chunks=[(9, 16), (25, 16)])
            emit_signs(0, s, split=True)
            for tri in range(3):
                emit_wtrans_f32(0, 0, tri)
            for tri in range(3):
                emit_wtrans_f32(0, 1, tri)
            emit_warm(1)
            emit_wdma(1, 0)
            s = emit_loads(0, chunks=[(41, 8)])
            emit_signs(0, s)
            emit_wdma(1, 1)
            s = emit_loads(0, chunks=[(49, 7)])
            emit_signs(0, s)
            # img0 mt0 row-groups with mt1 weight prep interleaved to
            # match the mt1 DMA arrival; mt1-cg0 casts ride the scalar
            # engine (free of stores), cg1 casts the DVE
            ots0 = {}
            for ci, h0 in enumerate(range(0, H, CHUNK)):
                ot = opool.tile([P, 2, CHUNK, W], mybir.dt.float32, name="ot")
                ots0[h0] = ot
                emit_mm_group(0, h0, 0, ot)
                if ci == 4:
                    emit_wsign(1, 0)
                    emit_wsign(1, 1)
                elif ci == 5:
                    for tri in range(3):
                        emit_wtrans(1, 0, tri, cast_on_scalar=True)
                elif ci == 6:
                    for tri in range(3):
                        emit_wtrans(1, 1, tri)
            # img1 loads issue on the load ring after img0 + weights
            srcs1 = emit_loads(1)
            for h0 in range(0, H, CHUNK):
                emit_mm_group(0, h0, 1, ots0[h0])
                nc.sync.dma_start(
                    out=y_ap[0]
                    .rearrange("(mt c) h w -> c mt h w", mt=2)[
                        :, :, h0 : h0 + CHUNK, :
                    ],
                    in_=ots0[h0],
                )
                # img1 signs spread between copybacks so they never block
                # the scalar queue ahead of time-critical work
                k = h0 // CHUNK
                if k < 7 and k != 0:
                    pass
            emit_signs(1, srcs1)
            for img in range(1, IMG_PER_CORE):
                if img + 1 < IMG_PER_CORE:
                    srcs = emit_loads(img + 1)
                    emit_signs(img + 1, srcs)
                emit_mms(img)
    nc.compile()
    return nc


def kernel(x, weight, alpha, trace=False):
    global last_result
    x = np.ascontiguousarray(np.asarray(x, dtype=np.float32))
    weight = np.ascontiguousarray(np.asarray(weight, dtype=np.float32))
    alpha = np.ascontiguousarray(np.asarray(alpha, dtype=np.float32))

    nc = build_conv_kernel()
    in_maps = [
        {
            "x": np.ascontiguousarray(x[i * IMG_PER_CORE : (i + 1) * IMG_PER_CORE]),
            "w": weight,
            "alpha": alpha,
        }
        for i in range(N_CORES)
    ]
    res = run_bass_kernel_spmd(nc, in_maps, list(range(N_CORES)), trace=trace)
    last_result = res
    out = np.concatenate([res.results[i]["y"] for i in range(N_CORES)], axis=0)
    return out.astype(np.float32, copy=False)


# revision 20
# speedup vs baseline: 1.0265x; 1.0265x over previous
"""XNOR-Net conv2d kernel for Trainium2.

Computes conv2d(sign(x), sign(W), stride=1, pad=1) * alpha for
x:(32,256,56,56) f32, W:(256,256,3,3) f32, alpha:(256,1,1) f32.

Strategy: data-parallel over batch (4 images per core x 8 cores).
Per core, implicit GEMM on the PE array in fp8. sign(x) is +-1 in
fp8e4 (exact); sign(W) is represented as +-0.5 (one-pass compute:
(w>0) - 0.5), with the missing x2 folded into alpha. Products are
+-0.5, accumulated in fp32 PSUM -> half-integers, exact; the final
scale restores integers, so the result is bit-exact vs the reference.

sign(x) lives in SBUF as a zero-padded fp8 image
[128 part = C_in%128, 2 c-groups, 58 rows, 64 row-stride]. Each 3x3
tap is one DoubleRow matmul contracting all 256 input channels
(K = 128 partitions x 2 c-groups): lhsT [128, 2cg, 128co], rhs
[128, 2cg, 8 rows, 56 cols] (shifted window, N=448). 9 taps
accumulate into one PSUM bank; copyback applies 2*alpha.

v9 schedule (vs ~122-127us for earlier versions):
- Strict load priority via a single HWDGE ring: ALL loads (weights,
  x) ride the scalar-engine queue in exact FIFO priority order
  [w-mt0cg0, alpha, w-mt0cg1, x rows 0-8, x rows 9-40, w-mt1cg0,
  x rows 41-48, w-mt1cg1, x rows 49-55, img1...]. Splitting loads
  across the two HWDGE rings lets the SDMA engines round-robin
  ~1:1 between rings, which starves whichever matters now; one ring
  makes priority deterministic. ALL y stores ride the sync ring, so
  stores never queue behind loads and the ring is warm at the tail.
- Startup weight fast path: PE-transposes the raw fp32 weights the
  moment each DMA lands (no SBUF sign stage) and folds (w>0)-0.5
  into the PSUM->SBUF evacuation on the DVE. First real matmul at
  ~13us with the HAM clock gate already at 8/8 from warmup matmuls,
  and the PE never idles long enough to re-throttle.
- mt1 weight prep (bf16 sign->transpose->cast path) is interleaved
  into img0's mt0 row-groups to match its DMA arrival (~20-22us).
- Tail: the final row-group computes mt0, then mt1 as 6-row + 2-row
  PSUM groups; the last 57KB store issues on the already-spinning
  sync ring right after a 2-row copyback, cutting the post-last-
  matmul tail to ~3.5us (store completion receipt dominates).
"""

import sys

sys.path.insert(0, "/opt/trn_rl_repo")

import numpy as np

import concourse.bass as bass
import concourse.mybir as mybir
from concourse import bacc
from concourse.bass_utils import run_bass_kernel_spmd
from concourse.masks import make_identity
from concourse.tile import TileContext

P = 128
N_CORES = 8
N_IMG = 32
IMG_PER_CORE = N_IMG // N_CORES
C = 256
H = W = 56
HP = 58  # padded rows (0..57)
WS = 64  # row stride of padded buffer (cols 0..57 used, 58+ never read)
CHUNK = 8  # output rows per matmul tile -> N = 8*56 = 448
LCHUNK = 16  # max rows per x load DMA
# (row0, nrows) per load DMA for steady-state images
CHUNKS = [(0, 9), (9, 16), (25, 16), (41, 15)]
FP8 = mybir.dt.float8e4

last_result = None  # stash of BassKernelResults for test harnesses


def build_conv_kernel():
    nc = bacc.Bacc()
    x_in = nc.declare_dram_parameter(
        "x", [IMG_PER_CORE, C, H, W], mybir.dt.float32, isOutput=False
    )
    w_in = nc.declare_dram_parameter("w", [C, C, 3, 3], mybir.dt.float32, isOutput=False)
    a_in = nc.declare_dram_parameter("alpha", [C, 1, 1], mybir.dt.float32, isOutput=False)
    y_out = nc.declare_dram_parameter(
        "y", [IMG_PER_CORE, C, H, W], mybir.dt.float32, isOutput=True
    )
    x_ap, w_ap, a_ap, y_ap = x_in[:], w_in[:], a_in[:], y_out[:]

    with TileContext(nc) as tc:
        with (
            tc.tile_pool(name="wpool", bufs=1) as wpool,
            tc.tile_pool(name="xpool", bufs=3) as xpool,
            tc.tile_pool(name="opool", bufs=8) as opool,
            tc.tile_pool(name="pp", bufs=4, space="PSUM") as pp,
        ):
            # PE prewarm: matmuls over zeros, issued before any real
            # dependency, so the clock gate is at 8/8 when weights land
            warm_rhs = wpool.tile([P, 512], FP8, name="warm_rhs")
            nc.vector.memset(warm_rhs, 0.0)
            warm_acc = pp.tile([P, 512], mybir.dt.float32, name="warm_acc", bufs=1)

            def emit_warm(n):
                for _ in range(n):
                    nc.tensor.matmul(
                        warm_acc, warm_rhs[:, 0:P], warm_rhs, start=True, stop=True
                    )

            emit_warm(7)

            # warm up the ACT function table while the first DMAs run
            warm = wpool.tile([P, 1], mybir.dt.float32, name="warm")
            nc.vector.memset(warm, 0.0)
            nc.scalar.sign(warm, warm)

            ident_f32 = wpool.tile([P, P], mybir.dt.float32, name="ident_f32")
            make_identity(nc, ident_f32)
            ident = wpool.tile([P, P], mybir.dt.bfloat16, name="ident")
            make_identity(nc, ident)
            alpha_sb = wpool.tile([P, 2], mybir.dt.float32, name="alpha_sb")

            # [ci_lo, cg, mt, pos, co]
            w_lhsT = wpool.tile([P, 2, 2, 9, P], FP8, name="w_lhsT")

            wsrcs = {}
            wsgns = {}

            def emit_wdma(mt, cg):
                wsrc = wpool.tile(
                    [P, P, 9], mybir.dt.float32, name=f"wsrc{mt}", bufs=2
                )
                nc.sync.dma_start(
                    out=wsrc,
                    in_=w_ap[
                        mt * P : (mt + 1) * P, cg * P : (cg + 1) * P
                    ].rearrange("co ci kh kw -> co ci (kh kw)"),
                )
                wsrcs[(mt, cg)] = wsrc

            def emit_wsign(mt, cg):
                # one-pass half-sign on DVE: (w > 0) - 0.5 -> +-0.5
                wsgn = wpool.tile(
                    [P, P, 9], mybir.dt.bfloat16, name=f"wsgn{mt}", bufs=2
                )
                nc.vector.tensor_scalar(
                    out=wsgn,
                    in0=wsrcs[(mt, cg)],
                    scalar1=0.0,
                    scalar2=0.5,
                    op0=mybir.AluOpType.is_gt,
                    op1=mybir.AluOpType.subtract,
                )
                wsgns[(mt, cg)] = wsgn

            def emit_wtrans(mt, cg, tri, cast_on_scalar=False):
                # transpose taps 3*tri..3*tri+2 into one PSUM tile, then a
                # single cast moves all three into the fp8 lhsT
                tp = pp.tile([P, 3, P], mybir.dt.bfloat16, name="tp", bufs=2)
                for k in range(3):
                    nc.tensor.transpose(
                        tp[:, k, :], wsgns[(mt, cg)][:, :, 3 * tri + k], ident
                    )
                dst = w_lhsT[:, cg, mt, 3 * tri : 3 * tri + 3, :]
                if cast_on_scalar:
                    nc.scalar.copy(out=dst, in_=tp)
                else:
                    nc.vector.tensor_copy(out=dst, in_=tp)

            def emit_wtrans_f32(mt, cg, tri):
                # startup fast path: PE-transpose the raw fp32 weights the
                # moment their DMA lands (no wsgn stage), and fold the
                # half-sign (w>0)-0.5 into the PSUM->SBUF evacuation
                tpf = pp.tile([P, 3, P], mybir.dt.float32, name="tpf", bufs=2)
                for k in range(3):
                    nc.tensor.transpose(
                        tpf[:, k, :], wsrcs[(mt, cg)][:, :, 3 * tri + k], ident_f32
                    )
                nc.vector.tensor_scalar(
                    out=w_lhsT[:, cg, mt, 3 * tri : 3 * tri + 3, :],
                    in0=tpf,
                    scalar1=0.0,
                    scalar2=0.5,
                    op0=mybir.AluOpType.is_gt,
                    op1=mybir.AluOpType.subtract,
                )

            xpads = {}

            def emit_xpad(img):
                xpad = xpool.tile([P, 2, HP, WS], FP8, name="xpad")
                xpads[img] = xpad
                nc.vector.memset(xpad[:, :, 0, 0:58], 0.0)
                nc.vector.memset(xpad[:, :, HP - 1, 0:58], 0.0)
                nc.vector.memset(xpad[:, :, 1 : HP - 1, 0], 0.0)
                nc.vector.memset(xpad[:, :, 1 : HP - 1, 57], 0.0)

            def emit_loads(img, chunks=None):
                # all x loads ride the sync (load) ring: the sync engine
                # runs no compute ops, so DMA issues never sit behind
                # signs/casts (head-of-line) and priority order is exact
                if chunks is None:
                    emit_xpad(img)
                    chunks = CHUNKS
                srcs = []
                for r0, rows in chunks:
                    for cg in range(2):
                        xsrc = xpool.tile(
                            [P, LCHUNK, W], mybir.dt.float32, name="xsrc", bufs=10
                        )
                        nc.sync.dma_start(
                            out=xsrc[:, 0:rows, :],
                            in_=x_ap[img, cg * P : (cg + 1) * P, r0 : r0 + rows],
                        )
                        srcs.append((r0, rows, cg, xsrc))
                return srcs

            def emit_signs(img, srcs, split=False):
                # sign on the ACT engine; split=True signs a 16-row chunk
                # as two 8-row ops so a row-group never waits on rows it
                # doesn't need yet
                xpad = xpads[img]
                for r0, rows, cg, xsrc in srcs:
                    pieces = (
                        [(0, rows // 2), (rows // 2, rows - rows // 2)]
                        if split and rows > 8
                        else [(0, rows)]
                    )
                    for p0, pr in pieces:
                        nc.scalar.sign(
                            xpad[
                                :, cg, r0 + p0 + 1 : r0 + p0 + 1 + pr, 1 : W + 1
                            ],
                            xsrc[:, p0 : p0 + pr, :],
                        )

            def emit_mm_group(img, h0, mt, ot, r0=0, nrows=CHUNK):
                # h0: absolute first output row; result rows land in
                # ot[:, mt, r0:r0+nrows]
                xpad = xpads[img]
                acc = pp.tile([P, nrows * W], mybir.dt.float32, name="acc", bufs=3)
                k = 0
                for kh in range(3):
                    for kw in range(3):
                        nc.tensor.matmul(
                            acc,
                            w_lhsT[:, :, mt, kh * 3 + kw, :],
                            xpad[:, :, h0 + kh : h0 + kh + nrows, kw : kw + W],
                            start=(k == 0),
                            stop=(k == 8),
                            perf_mode=mybir.MatmulPerfMode.DoubleRow,
                        )
                        k += 1
                # x2 restores the +-0.5 weight scale
                nc.vector.tensor_scalar(
                    out=ot[:, mt, r0 : r0 + nrows],
                    in0=acc.rearrange("p (r c) -> p r c", c=W),
                    scalar1=alpha_sb[:, mt : mt + 1],
                    scalar2=2.0,
                    op0=mybir.AluOpType.mult,
                    op1=mybir.AluOpType.mult,
                )

            def emit_row_group(img, h0):
                # both output halves for rows h0..h0+8, then one store on
                # the sync (store) ring
                ot = opool.tile([P, 2, CHUNK, W], mybir.dt.float32, name="ot")
                ydst = y_ap[img].rearrange("(mt c) h w -> c mt h w", mt=2)[
                    :, :, h0 : h0 + CHUNK, :
                ]
                emit_mm_group(img, h0, 0, ot)
                emit_mm_group(img, h0, 1, ot)
                nc.scalar.dma_start(out=ydst, in_=ot)

            def emit_tail_group(img, h0):
                # final row-group: mt0 stored as soon as its copyback is
                # done; mt1 split into 6-row + 2-row PSUM groups so the
                # first store's completion receipt overlaps the last rows'
                # matmuls
                ot = opool.tile([P, 2, CHUNK, W], mybir.dt.float32, name="ot")
                ydst = y_ap[img].rearrange("(mt c) h w -> c mt h w", mt=2)[
                    :, :, h0 : h0 + CHUNK, :
                ]
                emit_mm_group(img, h0, 0, ot)
                nc.scalar.dma_start(out=ydst[:, 0:1], in_=ot[:, 0:1])
                emit_mm_group(img, h0, 1, ot, r0=0, nrows=6)
                nc.scalar.dma_start(out=ydst[:, 1:2, 0:6], in_=ot[:, 1:2, 0:6])
                emit_mm_group(img, h0 + 6, 1, ot, r0=6, nrows=2)
                nc.scalar.dma_start(out=ydst[:, 1:2, 6:8], in_=ot[:, 1:2, 6:8])

            def emit_mms(img):
                last_img = img == IMG_PER_CORE - 1
                for h0 in range(0, H, CHUNK):
                    if last_img and h0 == H - CHUNK:
                        emit_tail_group(img, h0)
                    else:
                        emit_row_group(img, h0)

            # ---- startup: one load ring, strict priority order ----
            with tc.high_priority():
                emit_wdma(0, 0)
                nc.sync.dma_start(
                    out=alpha_sb,
                    in_=a_ap.flatten().rearrange("(mt co) -> co mt", co=P),
                )
                emit_wdma(0, 1)
            emit_xpad(0)
            s = emit_loads(0, chunks=[(0, 9)])
            emit_signs(0, s)
            s = emit_loads(0, chunks=[(9, 16), (25, 16)])
            emit_signs(0, s, split=True)
            for tri in range(3):
                emit_wtrans_f32(0, 0, tri)
            for tri in range(3):
                emit_wtrans_f32(0, 1, tri)
            emit_warm(1)
            emit_wdma(1, 0)
            s = emit_loads(0, chunks=[(41, 8)])
            emit_signs(0, s)
            emit_wdma(1, 1)
            s = emit_loads(0, chunks=[(49, 7)])
            emit_signs(0, s)
            # img0 mt0 row-groups with mt1 weight prep interleaved to
            # match the mt1 DMA arrival; mt1-cg0 casts ride the scalar
            # engine (free of stores), cg1 casts the DVE
            ots0 = {}
            for ci, h0 in enumerate(range(0, H, CHUNK)):
                ot = opool.tile([P, 2, CHUNK, W], mybir.dt.float32, name="ot")
                ots0[h0] = ot
                emit_mm_group(0, h0, 0, ot)
                if ci == 3:
                    emit_wsign(1, 0)
                elif ci == 4:
                    emit_wsign(1, 1)
                elif ci == 5:
                    for tri in range(3):
                        emit_wtrans(1, 0, tri, cast_on_scalar=True)
                    for tri in range(3):
                        emit_wtrans(1, 1, tri)
            # img1 loads issue on the load ring after img0 + weights
            srcs1 = emit_loads(1)
            for h0 in range(0, H, CHUNK):
                emit_mm_group(0, h0, 1, ots0[h0])
                nc.scalar.dma_start(
                    out=y_ap[0]
                    .rearrange("(mt c) h w -> c mt h w", mt=2)[
                        :, :, h0 : h0 + CHUNK, :
                    ],
                    in_=ots0[h0],
                )
            emit_signs(1, srcs1)
            for img in range(1, IMG_PER_CORE):
                if img + 1 < IMG_PER_CORE:
                    srcs = emit_loads(img + 1)
                    emit_signs(img + 1, srcs)
                emit_mms(img)
    nc.compile()
    return nc


def kernel(x, weight, alpha, trace=False):
    global last_result
    x = np.ascontiguousarray(np.asarray(x, dtype=np.float32))
    weight = np.ascontiguousarray(np.asarray(weight, dtype=np.float32))
    alpha = np.ascontiguousarray(np.asarray(alpha, dtype=np.float32))

    nc = build_conv_kernel()
    in_maps = [
        {
            "x": np.ascontiguousarray(x[i * IMG_PER_CORE : (i + 1) * IMG_PER_CORE]),
            "w": weight,
            "alpha": alpha,
        }
        for i in range(N_CORES)
    ]
    res = run_bass_kernel_spmd(nc, in_maps, list(range(N_CORES)), trace=trace)
    last_result = res
    out = np.concatenate([res.results[i]["y"] for i in range(N_CORES)], axis=0)
    return out.astype(np.float32, copy=False)
